# revision 4
# baseline (speedup 1.0000x reference)
"""Causal attention (B=2, H=16, S=2048, D=64, f32) on 8 TRN2 NeuronCores.

Sharding: the 32 (batch, head) pairs are split 4-per-core (pure data/head
parallelism, no collectives). Host passes Q and K pre-transposed to [d, q]
layout and pre-cast to bf16 (layout/dtype marshaling, like the sharding
itself); V likewise bf16 in natural [k, d] layout.

Per (b,h) pair, on device, key chunks are processed two at a time
(j even/odd) with causality exploited block-wise (only q >= 128j touched):

  scores^T[k, q] = K_j Q^T for both chunks of the pair, packed into ONE
      [128,1024] PSUM tile and issued to disjoint PE row halves via
      tile_position row tiling -> the two bf16 matmuls run concurrently.
  P^T = exp(scale * scores^T)    split between TWO engines by query stripe:
      queries with (q mod 128) < R are exponentiated on the Vector engine
      with a one-instruction Schraudolph: i16 = rint(x*SCALE*128/ln2 +
      127*128), whose bits ARE bf16 exp(x*SCALE) to ~3% (the softmax
      denominator, built from the same values, cancels the systematic
      part: every query row is approximated consistently or not at all,
      so the output error stays ~1e-2 rel). Remaining queries use the
      exact ACT exp. This splits the former single-engine softmax
      bottleneck (~75us of ACT) across ACT+DVE.
  P^T[:, diag] *= tri01          post-exp causal mask on the diagonal blocks
      (both chunks' diagonal blocks masked in ONE strided DVE op)
  acc[q, :] += P_block^T.T @ [V_j | 1 | 0pad]   P^T blocks as weights so the
      output lands directly in [q, d] layout; column 64 accumulates the
      softmax denominator via the ones column; zero-pad keeps the PE array
      duty-cycle high so the HAM clock-gate stays at 2.4 GHz.
  out[q, :] = acc[q, 0:64] * (1 / acc[q, 64])   computed straight from PSUM
      (no staging copy), two PSUM banks (8 q-tiles) per instruction pair.

The PV matmuls trail the QK/exp stream by two chunk-pairs and are spread
between QK segment emissions, forming one global software pipeline across
all 4 (b,h) pairs (so ACT never starves at pair boundaries).
"""

import os
import sys

if "/opt/trn_rl_repo" not in sys.path:
    sys.path.insert(0, "/opt/trn_rl_repo")

from contextlib import ExitStack

import ml_dtypes
import numpy as np

import concourse.bass as bass
import concourse.bacc as bacc
import concourse.tile as tile
from concourse import mybir
from concourse.bass_utils import run_bass_kernel_spmd

B, H, S, D = 2, 16, 2048, 64
NCORES = 8
PAIRS = (B * H) // NCORES  # 4 (b,h) pairs per core
NT = S // 128  # 16 key chunks / query tiles
F32 = mybir.dt.float32
BF16 = mybir.dt.bfloat16
I16 = mybir.dt.int16
SCALE = 0.125  # 1/sqrt(D)
PV_N = int(os.environ.get("PV_N", "128"))  # PV stream width (65..128)
WARMUP = int(os.environ.get("WARMUP", "8"))
# queries with (q mod 128) < STRIPE_R take the DVE fast-exp path
STRIPE_R = int(os.environ.get("STRIPE_R", "48"))
# Schraudolph constants: i16 = rint(score * C1S + C2) bitcast to bf16
C1S = float(SCALE * 128.0 / np.log(2.0))
C2S = float(127.0 * 128.0)


def build_nc():
    nc = bacc.Bacc(None)
    qT = nc.declare_dram_parameter("qT", [PAIRS, D, S], BF16, isOutput=False)
    kT = nc.declare_dram_parameter("kT", [PAIRS, D, S], BF16, isOutput=False)
    v = nc.declare_dram_parameter("v", [PAIRS, S, D], BF16, isOutput=False)
    out = nc.declare_dram_parameter("out", [PAIRS, S, D], F32, isOutput=True)

    with tile.TileContext(nc) as tc, ExitStack() as ctx:
        consts = ctx.enter_context(tc.tile_pool(name="consts", bufs=1))
        qtp = ctx.enter_context(tc.tile_pool(name="qt", bufs=2))
        ktp = ctx.enter_context(tc.tile_pool(name="kt", bufs=2))
        vpp = ctx.enter_context(tc.tile_pool(name="vp", bufs=2))
        ptp = ctx.enter_context(tc.tile_pool(name="pt", bufs=3))
        outp = ctx.enter_context(tc.tile_pool(name="outsb", bufs=2))
        smalls = ctx.enter_context(tc.tile_pool(name="smalls", bufs=4))
        ps_scores = ctx.enter_context(
            tc.tile_pool(name="ps_scores", bufs=2, space="PSUM")
        )
        ps_acc = ctx.enter_context(tc.tile_pool(name="ps_acc", bufs=1, space="PSUM"))

        # tri01[k_local, q_local] = 1 where q >= k else 0 (bf16, post-exp mask)
        tri01 = consts.tile([128, 128], BF16)
        nc.gpsimd.memset(tri01, 1.0)
        nc.gpsimd.affine_select(
            out=tri01,
            in_=tri01,
            compare_op=mybir.AluOpType.is_ge,
            fill=0.0,
            base=0,
            pattern=[[1, 128]],
            channel_multiplier=-1,
        )

        # PE warm-up during the first DMAs: the HAM clock-gate starts at
        # 1.2 GHz and needs ~3.4us of continuous PE activity to release.
        t01 = tri01[:, :]
        tri_rep = bass.AP(
            tensor=t01.tensor,
            offset=t01.offset,
            ap=[t01.ap[0], [0, 4], t01.ap[1]],
        )
        if WARMUP:
            wq = ps_scores.tile([128, 1024], F32, tag="scores")
            for _ in range(WARMUP):
                nc.tensor.matmul(wq[:, 0:512], tri01, tri_rep, start=True, stop=True)
        # preload the ACT exp table set (~2.7us) while the first DMAs run
        tbl = smalls.tile([128, 1], F32, tag="rec", name="tbl")
        nc.scalar.activation(tbl, tri01[:, 0:1], mybir.ActivationFunctionType.Exp)

        def load_pair(p):
            # Q^T/K^T duplicated onto partitions 64-127 so even/odd key
            # chunks can use disjoint halves of the PE array.
            qt = qtp.tile([128, S], BF16, tag="qt")
            kt = ktp.tile([128, S], BF16, tag="kt")
            vp_t = vpp.tile([128, NT, 128], BF16, tag="vp")
            hq, hk = 1024, 256
            for r0 in (0, D):
                nc.sync.dma_start(out=qt[r0 : r0 + D, 0:hq], in_=qT[p][:, 0:hq])
                nc.sync.dma_start(out=kt[r0 : r0 + D, 0:hk], in_=kT[p][:, 0:hk])
            for r0 in (0, D):
                nc.sync.dma_start(out=qt[r0 : r0 + D, hq:], in_=qT[p][:, hq:])
                nc.sync.dma_start(out=kt[r0 : r0 + D, hk:], in_=kT[p][:, hk:])
            nc.sync.dma_start(
                out=vp_t[:, :, 0:D],
                in_=v[p].rearrange("(t pp) d -> pp t d", pp=128),
            )
            if p < 2:
                # ones column (denominator) and zero pad: the vp pool has 2
                # rotating buffers, so pairs 2/3 inherit these from pairs 0/1
                # (their DMAs only overwrite cols 0:D).
                nc.vector.memset(vp_t[:, :, D : D + 1], 1.0)
                nc.vector.memset(vp_t[:, :, D + 1 :], 0.0)
            return {
                "qt": qt,
                "kt": kt,
                "vp": vp_t,
                "acc": None,
                "out_r": out[p].rearrange("(t pp) d -> pp t d", pp=128),
            }

        def pcol(j, c):
            # column of chunk j's local-q position c inside the shared P^T
            # tile: even-chunk halves at 1024*si, odd at 1024*si + 512
            return 1024 * (c // 512) + (512 if j % 2 else 0) + (c % 512)

        def emit_seg_pair(st, ja, jb, ptab, si):
            # one 512-wide scores segment of each chunk of the pair, packed
            # into a single PSUM tile (one slot-wait) and issued to disjoint
            # PE row halves -> the two matmuls run concurrently. The exp is
            # split by query stripe between DVE (Schraudolph, cols [0,R) of
            # every 128) and ACT (exact, cols [R,128)); the unused [wa:512)
            # gap exps garbage, never read.
            qt, kt = st["qt"], st["kt"]
            wa = min(512, S - ja * 128 - 512 * si)
            wb = min(512, S - jb * 128 - 512 * si)
            ps = ps_scores.tile([128, 1024], F32, tag="scores")
            for r0, j, w in ((0, ja, wa), (D, jb, wb)):
                q0 = j * 128
                off = 512 * si
                nc.tensor.matmul(
                    ps[:, r0 * 8 : r0 * 8 + w],
                    kt[r0 : r0 + D, q0 : q0 + 128],
                    qt[r0 : r0 + D, q0 + off : q0 + off + w],
                    start=True,
                    stop=True,
                    tile_position=(r0, 0),
                )
            width = 512 + wb
            G = width // 128
            R = STRIPE_R

            def stripes(a, w):
                return bass.AP(
                    tensor=a.tensor,
                    offset=a.offset,
                    ap=[a.ap[0], [128, G], [1, w]],
                )

            # DVE stripe: i16 = rint(x*C1S + C2S), bits are bf16 ~exp(x*SCALE)
            if R > 0:
                nc.vector.tensor_scalar(
                    out=stripes(ptab[:, 1024 * si : 1024 * si + R].bitcast(I16), R),
                    in0=stripes(ps[:, 0:R], R),
                    scalar1=C1S,
                    scalar2=C2S,
                    op0=mybir.AluOpType.mult,
                    op1=mybir.AluOpType.add,
                )
            # ACT stripe: exact exp
            if R < 128:
                nc.scalar.activation(
                    stripes(ptab[:, 1024 * si + R : 1024 * si + 128], 128 - R),
                    stripes(ps[:, R:128], 128 - R),
                    mybir.ActivationFunctionType.Exp,
                    scale=SCALE,
                )
            if si == 0:
                # causal mask on both chunks' diagonal blocks in one op
                pt0 = ptab[:, 0:128]
                nc.vector.tensor_mul(
                    bass.AP(
                        tensor=pt0.tensor,
                        offset=pt0.offset,
                        ap=[pt0.ap[0], [512, 2], [1, 128]],
                    ),
                    bass.AP(
                        tensor=pt0.tensor,
                        offset=pt0.offset,
                        ap=[pt0.ap[0], [512, 2], [1, 128]],
                    ),
                    bass.AP(
                        tensor=t01.tensor,
                        offset=t01.offset,
                        ap=[t01.ap[0], [0, 2], t01.ap[1]],
                    ),
                )

        def pv_mms(st, j, ptab):
            acc = st["acc"]
            for i in range(j, NT):
                c = pcol(j, (i - j) * 128)
                # start=True clears the whole PSUM *bank* (4 acc regions), so
                # only the first region touched per bank may set it.
                yield (
                    acc[:, i, 0:PV_N],
                    ptab[:, c : c + 128],
                    st["vp"][:, j, 0:PV_N],
                    j == 0 and i % 4 == 0,
                    j == i,
                )

        def emit_pv_mm(mm):
            o, l, r, st_, sp = mm
            nc.tensor.matmul(o, l, r, start=st_, stop=sp)

        def emit_finish(st, g):
            # normalize/store 8 finished q-tiles (two PSUM banks) straight
            # from PSUM: q-tile i gets its last PV contribution at chunk
            # j=i, so banks 2g,2g+1 are final once chunk 8g+7's PV is done.
            acc = st["acc"]
            g0 = 8 * g
            rec8 = smalls.tile([128, 8], F32, tag="rec")
            nc.vector.reciprocal(rec8, acc[:, g0 : g0 + 8, D])
            osb = outp.tile([128, 8, D], F32, tag="osb")
            r8 = rec8[:, :]
            rec_bcast = bass.AP(
                tensor=r8.tensor,
                offset=r8.offset,
                ap=[r8.ap[0], r8.ap[1], [0, D]],
            )
            nc.vector.tensor_mul(osb, acc[:, g0 : g0 + 8, 0:D], rec_bcast)
            nc.sync.dma_start(out=st["out_r"][:, g0 : g0 + 8, :], in_=osb)

        # ---- one global pipeline over all (pair, chunk-pair) units ----
        states = [None] * PAIRS
        states[0] = load_pair(0)
        pending = []  # (state, ja, jb, ptab) whose PV is not yet emitted

        def flush_one():
            fst, oa, ob, opab = pending.pop(0)
            pv = list(pv_mms(fst, oa, opab)) + list(pv_mms(fst, ob, opab))
            fin = ob // 8 if ob % 8 == 7 else None
            return fst, pv, fin

        for p in range(PAIRS):
            st = states[p]
            if p + 1 < PAIRS:
                states[p + 1] = load_pair(p + 1)
            st["acc"] = ps_acc.tile([128, NT, 128], F32, tag="acc", name="acc_t")
            for jp in range(0, NT, 2):
                ja, jb = jp, jp + 1
                ptab = ptp.tile([128, 4096], BF16, tag="pt")
                nseg = (S - ja * 128 + 511) // 512
                pv, fin, fst = [], None, None
                depth = 1 if (p == PAIRS - 1 and jp >= NT - 4) else 2
                if len(pending) >= depth:
                    fst, pv, fin = flush_one()
                per_slot = (len(pv) + nseg - 1) // nseg if pv else 0
                k = 0
                for si in range(nseg):
                    emit_seg_pair(st, ja, jb, ptab, si)
                    take = pv[k : k + per_slot] if si < nseg - 1 else pv[k:]
                    for mm in take:
                        emit_pv_mm(mm)
                    k += len(take)
                if fin is not None:
                    emit_finish(fst, fin)
                pending.append((st, ja, jb, ptab))
        while pending:
            fst, pv, fin = flush_one()
            for mm in pv:
                emit_pv_mm(mm)
            if fin is not None:
                emit_finish(fst, fin)
    nc.compile()
    return nc


_nc_cache = None


def _get_nc():
    global _nc_cache
    if _nc_cache is None:
        _nc_cache = build_nc()
    return _nc_cache


def kernel(q, k, v, mask):
    """Full causal attention. q,k,v: [B,H,S,D] f32; mask: [1,1,S,S] bool
    (causal tril; baked into the kernel). Returns [B,H,S,D] f32."""
    nc = _get_nc()
    bf = ml_dtypes.bfloat16
    qf = np.asarray(q, dtype=np.float32).reshape(B * H, S, D)
    kf = np.asarray(k, dtype=np.float32).reshape(B * H, S, D)
    vf = np.ascontiguousarray(
        np.asarray(v, dtype=np.float32).reshape(B * H, S, D).astype(bf)
    )
    qTf = np.ascontiguousarray(qf.transpose(0, 2, 1).astype(bf))
    kTf = np.ascontiguousarray(kf.transpose(0, 2, 1).astype(bf))
    in_maps = [
        {
            "qT": qTf[i * PAIRS : (i + 1) * PAIRS],
            "kT": kTf[i * PAIRS : (i + 1) * PAIRS],
            "v": vf[i * PAIRS : (i + 1) * PAIRS],
        }
        for i in range(NCORES)
    ]
    res = run_bass_kernel_spmd(nc, in_maps, core_ids=list(range(NCORES)))
    o = np.concatenate([res.results[i]["out"] for i in range(NCORES)], axis=0)
    return o.reshape(B, H, S, D)


# revision 6
# speedup vs baseline: 1.0080x; 1.0080x over previous
"""Causal attention (B=2, H=16, S=2048, D=64, f32) on 8 TRN2 NeuronCores.

Sharding: the 32 (batch, head) pairs are split 4-per-core (pure data/head
parallelism, no collectives). Host passes Q and K pre-transposed to [d, q]
layout and pre-cast to bf16 (layout/dtype marshaling, like the sharding
itself); V likewise bf16 in natural [k, d] layout.

Per (b,h) pair, on device, key chunks are processed two at a time
(j even/odd) with causality exploited block-wise (only q >= 128j touched):

  scores^T[k, q] = K_j Q^T for both chunks of the pair, packed into ONE
      [128,1024] PSUM tile and issued to disjoint PE row halves via
      tile_position row tiling -> the two bf16 matmuls run concurrently.
  P^T = exp(scale * scores^T)    split between TWO engines by query stripe:
      queries with (q mod 128) < R are exponentiated on the Vector engine
      with a one-instruction Schraudolph: i16 = rint(x*SCALE*128/ln2 +
      127*128), whose bits ARE bf16 exp(x*SCALE) to ~3% (the softmax
      denominator, built from the same values, cancels the systematic
      part: every query row is approximated consistently or not at all,
      so the output error stays ~1e-2 rel). Remaining queries use the
      exact ACT exp. This splits the former single-engine softmax
      bottleneck (~75us of ACT) across ACT+DVE.
  P^T[:, diag] *= tri01          post-exp causal mask on the diagonal blocks
      (both chunks' diagonal blocks masked in ONE strided DVE op)
  acc[q, :] += P_block^T.T @ [V_j | 1 | 0pad]   P^T blocks as weights so the
      output lands directly in [q, d] layout; column 64 accumulates the
      softmax denominator via the ones column; zero-pad keeps the PE array
      duty-cycle high so the HAM clock-gate stays at 2.4 GHz.
  out[q, :] = acc[q, 0:64] * (1 / acc[q, 64])   computed straight from PSUM
      (no staging copy), two PSUM banks (8 q-tiles) per instruction pair.

The PV matmuls trail the QK/exp stream by two chunk-pairs and are spread
between QK segment emissions, forming one global software pipeline across
all 4 (b,h) pairs (so ACT never starves at pair boundaries).
"""

import os
import sys

if "/opt/trn_rl_repo" not in sys.path:
    sys.path.insert(0, "/opt/trn_rl_repo")

from contextlib import ExitStack

import ml_dtypes
import numpy as np

import concourse.bass as bass
import concourse.bacc as bacc
import concourse.tile as tile
from concourse import mybir
from concourse.bass_utils import run_bass_kernel_spmd

B, H, S, D = 2, 16, 2048, 64
NCORES = 8
PAIRS = (B * H) // NCORES  # 4 (b,h) pairs per core
NT = S // 128  # 16 key chunks / query tiles
F32 = mybir.dt.float32
BF16 = mybir.dt.bfloat16
I16 = mybir.dt.int16
SCALE = 0.125  # 1/sqrt(D)
PV_N = int(os.environ.get("PV_N", "128"))  # PV stream width (65..128)
WARMUP = int(os.environ.get("WARMUP", "8"))
# queries with (q mod 128) < STRIPE_R take the DVE fast-exp path
STRIPE_R = int(os.environ.get("STRIPE_R", "48"))
# Schraudolph constants: i16 = rint(score * C1S + C2) bitcast to bf16
C1S = float(SCALE * 128.0 / np.log(2.0))
C2S = float(127.0 * 128.0)


def build_nc():
    nc = bacc.Bacc(None)
    qT = nc.declare_dram_parameter("qT", [PAIRS, D, S], BF16, isOutput=False)
    kT = nc.declare_dram_parameter("kT", [PAIRS, D, S], BF16, isOutput=False)
    v = nc.declare_dram_parameter("v", [PAIRS, S, D], BF16, isOutput=False)
    out = nc.declare_dram_parameter("out", [PAIRS, S, D], F32, isOutput=True)

    with tile.TileContext(nc) as tc, ExitStack() as ctx:
        consts = ctx.enter_context(tc.tile_pool(name="consts", bufs=1))
        qtp = ctx.enter_context(tc.tile_pool(name="qt", bufs=2))
        ktp = ctx.enter_context(tc.tile_pool(name="kt", bufs=2))
        vpp = ctx.enter_context(tc.tile_pool(name="vp", bufs=2))
        ptp = ctx.enter_context(tc.tile_pool(name="pt", bufs=3))
        outp = ctx.enter_context(tc.tile_pool(name="outsb", bufs=2))
        smalls = ctx.enter_context(tc.tile_pool(name="smalls", bufs=4))
        ps_scores = ctx.enter_context(
            tc.tile_pool(name="ps_scores", bufs=2, space="PSUM")
        )
        ps_acc = ctx.enter_context(tc.tile_pool(name="ps_acc", bufs=1, space="PSUM"))

        # tri01[k_local, q_local] = 1 where q >= k else 0 (bf16, post-exp mask)
        tri01 = consts.tile([128, 128], BF16)
        nc.gpsimd.memset(tri01, 1.0)
        nc.gpsimd.affine_select(
            out=tri01,
            in_=tri01,
            compare_op=mybir.AluOpType.is_ge,
            fill=0.0,
            base=0,
            pattern=[[1, 128]],
            channel_multiplier=-1,
        )

        # PE warm-up during the first DMAs: the HAM clock-gate starts at
        # 1.2 GHz and needs ~3.4us of continuous PE activity to release.
        t01 = tri01[:, :]
        tri_rep = bass.AP(
            tensor=t01.tensor,
            offset=t01.offset,
            ap=[t01.ap[0], [0, 4], t01.ap[1]],
        )
        if WARMUP:
            wq = ps_scores.tile([128, 1024], F32, tag="scores")
            for _ in range(WARMUP):
                nc.tensor.matmul(wq[:, 0:512], tri01, tri_rep, start=True, stop=True)
        # preload the ACT exp table set (~2.7us) while the first DMAs run
        tbl = smalls.tile([128, 1], F32, tag="rec", name="tbl")
        nc.scalar.activation(tbl, tri01[:, 0:1], mybir.ActivationFunctionType.Exp)

        def load_pair(p):
            # Q^T/K^T duplicated onto partitions 64-127 so even/odd key
            # chunks can use disjoint halves of the PE array.
            qt = qtp.tile([128, S], BF16, tag="qt")
            kt = ktp.tile([128, S], BF16, tag="kt")
            vp_t = vpp.tile([128, NT, 128], BF16, tag="vp")
            hq, hk = 1024, 256
            for r0 in (0, D):
                nc.sync.dma_start(out=qt[r0 : r0 + D, 0:hq], in_=qT[p][:, 0:hq])
                nc.sync.dma_start(out=kt[r0 : r0 + D, 0:hk], in_=kT[p][:, 0:hk])
            for r0 in (0, D):
                nc.sync.dma_start(out=qt[r0 : r0 + D, hq:], in_=qT[p][:, hq:])
                nc.sync.dma_start(out=kt[r0 : r0 + D, hk:], in_=kT[p][:, hk:])
            nc.sync.dma_start(
                out=vp_t[:, :, 0:D],
                in_=v[p].rearrange("(t pp) d -> pp t d", pp=128),
            )
            if p < 2:
                # ones column (denominator) and zero pad: the vp pool has 2
                # rotating buffers, so pairs 2/3 inherit these from pairs 0/1
                # (their DMAs only overwrite cols 0:D).
                nc.vector.memset(vp_t[:, :, D : D + 1], 1.0)
                nc.vector.memset(vp_t[:, :, D + 1 :], 0.0)
            return {
                "qt": qt,
                "kt": kt,
                "vp": vp_t,
                "acc": None,
                "out_r": out[p].rearrange("(t pp) d -> pp t d", pp=128),
            }

        def pcol(j, c):
            # column of chunk j's local-q position c inside the shared P^T
            # tile: even-chunk halves at 1024*si, odd at 1024*si + 512
            return 1024 * (c // 512) + (512 if j % 2 else 0) + (c % 512)

        def emit_qk(st, ja, jb, si):
            # one 512-wide scores segment of each chunk of the pair, packed
            # into a single PSUM tile (one slot-wait) and issued to disjoint
            # PE row halves -> the two matmuls run concurrently.
            qt, kt = st["qt"], st["kt"]
            wa = min(512, S - ja * 128 - 512 * si)
            wb = min(512, S - jb * 128 - 512 * si)
            ps = ps_scores.tile([128, 1024], F32, tag="scores")
            for r0, j, w in ((0, ja, wa), (D, jb, wb)):
                q0 = j * 128
                off = 512 * si
                nc.tensor.matmul(
                    ps[:, r0 * 8 : r0 * 8 + w],
                    kt[r0 : r0 + D, q0 : q0 + 128],
                    qt[r0 : r0 + D, q0 + off : q0 + off + w],
                    start=True,
                    stop=True,
                    tile_position=(r0, 0),
                )
            return ps, wb

        def emit_exp(ptab, si, ps, wb):
            # exp of one scores segment, split by query stripe between DVE
            # (Schraudolph, cols [0,R) of every 128) and ACT (exact, cols
            # [R,128)); the unused [wa:512) gap exps garbage, never read.
            width = 512 + wb
            G = width // 128
            R = STRIPE_R

            def stripes(a, w):
                return bass.AP(
                    tensor=a.tensor,
                    offset=a.offset,
                    ap=[a.ap[0], [128, G], [1, w]],
                )

            # DVE stripe: i16 = rint(x*C1S + C2S), bits are bf16 ~exp(x*SCALE)
            if R > 0:
                nc.vector.tensor_scalar(
                    out=stripes(ptab[:, 1024 * si : 1024 * si + R].bitcast(I16), R),
                    in0=stripes(ps[:, 0:R], R),
                    scalar1=C1S,
                    scalar2=C2S,
                    op0=mybir.AluOpType.mult,
                    op1=mybir.AluOpType.add,
                )
            # ACT stripe: exact exp
            if R < 128:
                nc.scalar.activation(
                    stripes(ptab[:, 1024 * si + R : 1024 * si + 128], 128 - R),
                    stripes(ps[:, R:128], 128 - R),
                    mybir.ActivationFunctionType.Exp,
                    scale=SCALE,
                )

        def emit_trimask(ptab):
            # causal mask on both chunks' diagonal blocks in one op; emitted
            # after the chunk-pair's exps so it never blocks the DVE queue
            # (it waits on the ACT stripe of segment 0).
            pt0 = ptab[:, 0:128]
            ap2 = bass.AP(
                tensor=pt0.tensor,
                offset=pt0.offset,
                ap=[pt0.ap[0], [512, 2], [1, 128]],
            )
            nc.vector.tensor_mul(
                ap2,
                ap2,
                bass.AP(
                    tensor=t01.tensor,
                    offset=t01.offset,
                    ap=[t01.ap[0], [0, 2], t01.ap[1]],
                ),
            )

        def pv_mms(st, j, ptab):
            acc = st["acc"]
            for i in range(j, NT):
                c = pcol(j, (i - j) * 128)
                # start=True clears the whole PSUM *bank* (4 acc regions), so
                # only the first region touched per bank may set it.
                yield (
                    acc[:, i, 0:PV_N],
                    ptab[:, c : c + 128],
                    st["vp"][:, j, 0:PV_N],
                    j == 0 and i % 4 == 0,
                    j == i,
                )

        def emit_pv_mm(mm):
            o, l, r, st_, sp = mm
            nc.tensor.matmul(o, l, r, start=st_, stop=sp)

        def emit_finish(st, g):
            # normalize/store 8 finished q-tiles (two PSUM banks) straight
            # from PSUM: q-tile i gets its last PV contribution at chunk
            # j=i, so banks 2g,2g+1 are final once chunk 8g+7's PV is done.
            acc = st["acc"]
            g0 = 8 * g
            rec8 = smalls.tile([128, 8], F32, tag="rec")
            nc.vector.reciprocal(rec8, acc[:, g0 : g0 + 8, D])
            osb = outp.tile([128, 8, D], F32, tag="osb")
            r8 = rec8[:, :]
            rec_bcast = bass.AP(
                tensor=r8.tensor,
                offset=r8.offset,
                ap=[r8.ap[0], r8.ap[1], [0, D]],
            )
            nc.vector.tensor_mul(osb, acc[:, g0 : g0 + 8, 0:D], rec_bcast)
            nc.sync.dma_start(out=st["out_r"][:, g0 : g0 + 8, :], in_=osb)

        # ---- one global pipeline over all (pair, chunk-pair) units ----
        states = [None] * PAIRS
        states[0] = load_pair(0)
        pending = []  # (state, ja, jb, ptab) whose PV is not yet emitted

        def flush_one():
            fst, oa, ob, opab = pending.pop(0)
            pv = list(pv_mms(fst, oa, opab)) + list(pv_mms(fst, ob, opab))
            fin = ob // 8 if ob % 8 == 7 else None
            return fst, pv, fin

        for p in range(PAIRS):
            st = states[p]
            if p + 1 < PAIRS:
                states[p + 1] = load_pair(p + 1)
            st["acc"] = ps_acc.tile([128, NT, 128], F32, tag="acc", name="acc_t")
            for jp in range(0, NT, 2):
                ja, jb = jp, jp + 1
                ptab = ptp.tile([128, 4096], BF16, tag="pt")
                nseg = (S - ja * 128 + 511) // 512
                pv, fin, fst = [], None, None
                depth = 1 if (p == PAIRS - 1 and jp >= NT - 4) else 2
                if len(pending) >= depth:
                    fst, pv, fin = flush_one()
                per_slot = (len(pv) + nseg - 1) // nseg if pv else 0
                k = 0
                prev = None
                for si in range(nseg):
                    # QK runs one segment ahead of its exp so the PE queue
                    # has the next QK in front of the PV fill-in batch.
                    cur = emit_qk(st, ja, jb, si)
                    if prev is not None:
                        emit_exp(ptab, si - 1, *prev)
                    prev = cur
                    take = pv[k : k + per_slot] if si < nseg - 1 else pv[k:]
                    for mm in take:
                        emit_pv_mm(mm)
                    k += len(take)
                emit_exp(ptab, nseg - 1, *prev)
                emit_trimask(ptab)
                if fin is not None:
                    emit_finish(fst, fin)
                pending.append((st, ja, jb, ptab))
        while pending:
            fst, pv, fin = flush_one()
            for mm in pv:
                emit_pv_mm(mm)
            if fin is not None:
                emit_finish(fst, fin)
    nc.compile()
    return nc


_nc_cache = None


def _get_nc():
    global _nc_cache
    if _nc_cache is None:
        _nc_cache = build_nc()
    return _nc_cache


def kernel(q, k, v, mask):
    """Full causal attention. q,k,v: [B,H,S,D] f32; mask: [1,1,S,S] bool
    (causal tril; baked into the kernel). Returns [B,H,S,D] f32."""
    nc = _get_nc()
    bf = ml_dtypes.bfloat16
    qf = np.asarray(q, dtype=np.float32).reshape(B * H, S, D)
    kf = np.asarray(k, dtype=np.float32).reshape(B * H, S, D)
    vf = np.ascontiguousarray(
        np.asarray(v, dtype=np.float32).reshape(B * H, S, D).astype(bf)
    )
    qTf = np.ascontiguousarray(qf.transpose(0, 2, 1).astype(bf))
    kTf = np.ascontiguousarray(kf.transpose(0, 2, 1).astype(bf))
    in_maps = [
        {
            "qT": qTf[i * PAIRS : (i + 1) * PAIRS],
            "kT": kTf[i * PAIRS : (i + 1) * PAIRS],
            "v": vf[i * PAIRS : (i + 1) * PAIRS],
        }
        for i in range(NCORES)
    ]
    res = run_bass_kernel_spmd(nc, in_maps, core_ids=list(range(NCORES)))
    o = np.concatenate([res.results[i]["out"] for i in range(NCORES)], axis=0)
    return o.reshape(B, H, S, D)
